# revision 35
# baseline (speedup 1.0000x reference)
"""BagOfWords Trainium2 kernel (fp8 DoubleRow pipeline).

Reference computation (per batch b):
    emb    = emb_table[context]                      # (T, D) gather
    logits = emb @ W.T + b                           # (T, V)
    out[t] = (sum_{s<=t} (s+1) * logits[s]) / den[t] # weighted causal cum-avg
    den[t] = (t+1)(t+2)/2

Key identity: the weighted cumsum commutes with the GEMM:
    out[t, v] = (num[t] @ W[v]) / den[t] + b[v]
    num[t, d] = sum_{s<=t} (s+1) * emb[s, d]
so the O(T*V) cumsum collapses onto the tiny (T, D) embedding side.
Per 128-token chunk (PE / DVE):
    psum[d, t] = sum_s emb[s, d] * UTW_c[s, t]       # prefix matmul per d-chunk
    NT[d, t]   = psum[d, t] + carry_prev[d]          # DVE copy w/ carry scalar
with the carry chain kept exact in fp32 (carry_sb) via paired DVE
tensor_tensor updates of the psum's last columns.

fp8 acceleration: the big GEMM out = NT.T @ W.T runs in fp8e4m3 with
MatmulPerfMode.DoubleRow -- K=256 per matmul at 0.5 cycles/moving-column,
2x the bf16 rate.  D=384 is zero-padded to 512 (2 DoubleRow matmuls/tile;
the 4th k-slice of both NT and W is zeroed).  Precision: the output's
global max lives at EARLY tokens (den[t] ~ t^2 makes late outputs tiny),
so chunk 0 of each batch stays on the bf16 path (bf16 NT, bf16 W, bf16
output) while chunks 1..7 use fp8 NT (per-chunk pow2 scale 2^-E_NT[c]),
fp8 W (2^EW), and fp8 *output* (per-chunk 2^S_OUT[c] folded into the
eviction constant, dequantized on host).  Measured end-to-end rel err
~5e-3 vs the fp32 reference (gate is 2e-2).

fp8 output also cuts the dominant HBM store traffic 2x (16.4 -> 9.2
MB/core total DMA ~16 MB ~ 44 us at 360 GB/s), and DoubleRow cuts PE time
80 -> 36 us.  PSUM->SBUF evictions (per-partition 1/den scale + dtype
convert) are the third resource: GPSIMD cannot touch PSUM, so they are
split between ACT and DVE in 2-tile (1000-column) pairs over 2-bank PSUM
tensors to amortize init overhead; DVE additionally owns the NT copies
((psum + carry)*2^-e in one two-scalar tensor_scalar op) and the paired
carry updates.

Sharding (8 cores): 4-way over B x 2-way over V.  Each core gathers 2
batches (2048 rows) and holds half of W (bf16 + fp8 copies).

Raw Bass with manual semaphores (one wait per instruction): the walrus build
in this container rejects instructions carrying multiple sem waits.

DMA semaphore discipline: a DMA's 16 per-SDMA-engine sem increments interleave
arbitrarily with other in-flight DMAs on the same semaphore, so every
concurrently-outstanding DMA group gets its own semaphore, waited to exactly
16 per iteration.

reps>1 repeats the whole pipeline inside one NEFF (used only for timing).
"""

import functools
import os
from contextlib import ExitStack

import numpy as np

import concourse.bass as bass
from concourse import mybir
from concourse.bass_utils import run_bass_kernel_spmd

B, T, V, D = 8, 1024, 8000, 384
P = 128
NCORE = 8
NCHUNK = T // P                 # 8 token chunks per batch
KD = D // P                     # 3 real contraction chunks
NV = 500                        # vocab tile (one fp32 PSUM bank half)
VGRP = 4                        # vocab tiles per store group
F32 = mybir.dt.float32
BF16 = mybir.dt.bfloat16
F8 = mybir.dt.float8e4
DR = mybir.MatmulPerfMode.DoubleRow
Alu = mybir.AluOpType

NVG = 2                         # vocab groups (cores split 4B x 2V)
WARM = int(os.environ.get("BOW_WARM", "0"))
NB = NVG                        # batches per core
V_CORE = V // NVG               # 4000 vocab columns per core
BT = NB * T                     # 2048 tokens per core
NCHT = NB * NCHUNK              # 16 token chunks per core
NTV = V_CORE // NV              # 8 vocab tiles per core
NGRP = NTV // VGRP              # 2 store column groups
GCOLS = VGRP * NV               # 2000 columns per weight/store group
NBLK = NGRP * NCHT              # 32 gemm blocks per iteration
GM_IT = NBLK * VGRP             # gemm tiles per iteration
CT_IT = NCHT * KD               # NT copies per iteration
NPAIR = GM_IT // 2              # eviction pairs per iteration (64)
NST8 = 8                        # fp8 staging buffers
NST16 = 3                       # bf16 staging buffers

# fp8 scale exponents (host-validated: global rel err ~5.4e-3)
EW = 6                                       # W8 = W * 2^EW
E_NT = [0, 6, 6, 7, 8, 8, 8, 9]              # NT8 = NT * 2^-E_NT[c]
S_OUT = [0, 11, 11, 11, 11, 12, 12, 12]      # out8 = out * 2^S_OUT[c]

# one single-chunk gather per 128 tokens (multi-chunk offset APs scramble
# the destination layout on real hardware)
GATHER_GROUPS = [1] * NCHT

# gemm block sweep order (see block_seq): fp8 chunks first, chunk-0s after
# their batch's fp8 run has started; invariant CHUNK_ORDER[j] <= j+1
CHUNK_ORDER = [1, 2, 3, 4, 5, 6, 7, 0, 9, 8, 10, 11, 12, 13, 14, 15]

# --- eviction pair -> engine assignment ---------------------------------
# Blocks sweep g-INNER ((0,mc),(1,mc),(0,mc+1),...) so each chunk's NT-copy
# work spreads over two block periods.  Block bi has pairs (2bi, 2bi+1).
# DVE owns the (fused) NT copies + carry chain (~0.79us/chunk), so ACT
# takes 5 of every 8 pairs: per 2 chunks ACT 5x1.02 = 5.09us vs DVE
# 3x1.17 + 2x0.79 = 5.08us.

_PAT = ["a", "d", "a", "a", "d", "a", "a", "d"]


def _pair_engine(q):
    return _PAT[q % len(_PAT)]

_A_IT = sum(1 for x in range(NPAIR) if _pair_engine(x) == "a")
_D_IT = NPAIR - _A_IT


def _pair_count(q):
    """1-based per-engine count of pair q among pairs of its engine."""
    e = _pair_engine(q)
    it, qq = divmod(q, NPAIR)
    base = (_A_IT if e == "a" else _D_IT) * it
    return base + sum(1 for x in range(qq + 1) if _pair_engine(x) == e)


def _block_evict_counts(it, bi):
    """Cumulative (asem, dsem) counts once block bi's pairs are evicted."""
    a = sum(1 for x in range(2 * bi + 2) if _pair_engine(x) == "a") + _A_IT * it
    d = sum(1 for x in range(2 * bi + 2) if _pair_engine(x) == "d") + _D_IT * it
    return a, d


def _build(has_bias: bool, reps: int = 1, dbg: bool = False):
    nc = bass.Bass("TRN2", target_bir_lowering=False, debug=False)

    idx_d = nc.dram_tensor("idx", [P, NCHT], mybir.dt.int32, kind="ExternalInput")
    table_d = nc.dram_tensor("table", [V, D], BF16, kind="ExternalInput")
    wt_d = nc.dram_tensor("wt", [D, V_CORE], BF16, kind="ExternalInput")
    wt8_d = nc.dram_tensor("wt8", [P, NGRP * KD * GCOLS], F8, kind="ExternalInput")
    cst16_d = nc.dram_tensor("cst16", [P, NCHUNK * P], BF16, kind="ExternalInput")
    cst32_d = nc.dram_tensor("cst32", [P, NCHUNK], F32, kind="ExternalInput")
    out16_d = nc.dram_tensor("out16", [NB * P, V_CORE], BF16, kind="ExternalOutput")
    out8_d = nc.dram_tensor("out8", [NB * (NCHUNK - 1) * P, V_CORE], F8,
                            kind="ExternalOutput")

    with ExitStack() as ctx:
        e = ctx.enter_context
        # SBUF
        idx_sb = e(nc.sbuf_tensor("idx_sb", [P, NCHT], mybir.dt.int32))
        cst16 = e(nc.sbuf_tensor("cst16_sb", [P, NCHUNK * P], BF16))
        cst32 = e(nc.sbuf_tensor("cst32_sb", [P, NCHUNK], F32))
        emb_sb = e(nc.sbuf_tensor("emb_sb", [P, NCHT * D], BF16))
        # bf16 NT: chunk 0 of each batch only
        ct16 = e(nc.sbuf_tensor("ct16", [P, KD, NB * P], BF16))
        # fp8 NT: [p, k-slice (4th zeroed), token]; chunk-0 columns unused
        ct8 = e(nc.sbuf_tensor("ct8", [P, 4, BT], F8))
        carry_sb = e(nc.sbuf_tensor("carry_sb", [P, KD * NCHT], F32))
        # pre-scaled carry columns (carry * 2^-e) for the fused NT copy
        carrysc_sb = e(nc.sbuf_tensor("carrysc_sb", [P, KD * NCHT], F32))
        wt_sb = [e(nc.sbuf_tensor(f"wt{k}", [P, V_CORE], BF16)) for k in range(KD)]
        wt8_sb = e(nc.sbuf_tensor("wt8_sb", [P, NGRP * KD, GCOLS], F8))
        ostg8 = [e(nc.sbuf_tensor(f"ostg8_{q}", [P, VGRP, NV], F8))
                 for q in range(NST8)]
        ostg16 = [e(nc.sbuf_tensor(f"ostg16_{q}", [P, VGRP, NV], BF16))
                  for q in range(NST16)]
        # PSUM: 3 x 2-bank gemm pair tensors + 2 x 1-bank prefix tensors
        gps = [e(nc.psum_tensor(f"gps{i}", [P, 2, 512], F32)) for i in range(3)]
        ctps = [e(nc.psum_tensor(f"ctps{i}", [P, KD * P], F32)) for i in range(2)]
        # sems
        gidx = e(nc.semaphore("gidx"))
        csem16 = e(nc.semaphore("csem16"))
        csem32 = e(nc.semaphore("csem32"))
        wsem16 = [e(nc.semaphore(f"wsem16_{g}")) for g in range(NGRP)]
        w8sem = [e(nc.semaphore(f"w8sem_{g}")) for g in range(NGRP)]
        k3sem = e(nc.semaphore("k3sem"))
        gsem = [e(nc.semaphore(f"gsem{gg}")) for gg in range(NCHT)]
        ctdone = e(nc.semaphore("ctdone"))      # prefix psum matmuls (PE)
        ctsbD = e(nc.semaphore("ctsbD"))        # fused NT copies (DVE)
        carrysem = e(nc.semaphore("carrysem"))  # carry updates (DVE)
        cssem = e(nc.semaphore("cssem"))        # scaled carry cols (DVE)
        pegemm = e(nc.semaphore("pegemm"))      # gemm tiles (PE)
        asem = e(nc.semaphore("asem"))          # ACT pair evictions
        dsem = e(nc.semaphore("dsem"))          # DVE pair evictions
        osem8 = [e(nc.semaphore(f"osem8_{q}")) for q in range(NST8)]
        osem16 = [e(nc.semaphore(f"osem16_{q}")) for q in range(NST16)]
        blk = e(nc.Block())

        utw_ap = lambda c: cst16[:, c * P:(c + 1) * P]
        evc_ap = lambda c: cst32[:, c:c + 1]

        # block order = gemm order: g-INNER ((0,mc),(1,mc),(0,mc'),...) over
        # CHUNK_ORDER, which runs the fp8 chunks FIRST (their weights are
        # 1.5 MB vs bf16's 3 MB, so the gemm starts ~15us earlier) and slots
        # each batch's bf16 chunk-0 in once wt16 has landed.  Legal because
        # the NT chain (natural order 0..15) is decoupled from block order:
        # CHUNK_ORDER[j] <= j+1 for all j.
        def block_seq():
            for mc in CHUNK_ORDER:
                for g in range(NGRP):
                    yield g, mc

        def is16(mc):
            return mc % NCHUNK == 0

        # staging rotation indices, per iteration-local block index
        _stg = {}
        n8 = n16 = 0
        for _bi, (_g, _mc) in enumerate(block_seq()):
            if is16(_mc):
                _stg[_bi] = ("16", n16)
                n16 += 1
            else:
                _stg[_bi] = ("8", n8)
                n8 += 1
        N8_IT, N16_IT = n8, n16      # 28 fp8 / 4 bf16 blocks per iter

        def stg_buf(it, bi):
            kind, n = _stg[bi]
            if kind == "8":
                gi = it * N8_IT + n
                return ostg8[gi % NST8], osem8[gi % NST8], gi, NST8
            gi = it * N16_IT + n
            return ostg16[gi % NST16], osem16[gi % NST16], gi, NST16

        @blk.sync
        def _(sync):
            # idx first: the gather chain is the critical path
            sync.dma_start(idx_sb[:], idx_d[:]).then_inc(gidx, 16)
            sync.dma_start(cst32[:], cst32_d[:]).then_inc(csem32, 16)
            sync.dma_start(cst16[:], cst16_d[:]).then_inc(csem16, 16)
            # weights in need order (fp8 blocks run first), in ~0.7us bands
            # so the gather transfers interleave on the DMA bus
            for g in range(NGRP):
                for k in range(KD):
                    sync.dma_start(
                        wt8_sb[:, g * KD + k:g * KD + k + 1, :],
                        wt8_d[:, (g * KD + k) * GCOLS:(g * KD + k + 1) * GCOLS]
                    ).then_inc(w8sem[g], 16)
            for g in range(NGRP):
                for k in range(KD):
                    for h in range(2):
                        c0 = g * GCOLS + h * (GCOLS // 2)
                        sync.dma_start(
                            wt_sb[k][:, c0:c0 + GCOLS // 2],
                            wt_d[k * P:(k + 1) * P, c0:c0 + GCOLS // 2]
                        ).then_inc(wsem16[g], 16)
            # output stores in gemm-block order
            for it in range(reps):
                for bi, (g, mc) in enumerate(block_seq()):
                    buf, sem, gi, nst = stg_buf(it, bi)
                    ac, dc = _block_evict_counts(it, bi)
                    sync.wait_ge(asem, ac)
                    if dc > 0:
                        sync.wait_ge(dsem, dc)
                    if is16(mc):
                        bt = mc // NCHUNK
                        sync.dma_start(
                            out16_d[bt * P:(bt + 1) * P,
                                    g * GCOLS:(g + 1) * GCOLS],
                            buf[:, :, :]).then_inc(sem, 16)
                    else:
                        bt, c = divmod(mc, NCHUNK)
                        f = bt * (NCHUNK - 1) + (c - 1)
                        sync.dma_start(
                            out8_d[f * P:(f + 1) * P,
                                   g * GCOLS:(g + 1) * GCOLS],
                            buf[:, :, :]).then_inc(sem, 16)
            for q in range(NST8):
                tot = (reps * N8_IT - q + NST8 - 1) // NST8
                if tot > 0:
                    sync.wait_ge(osem8[q], 16 * tot)
            for q in range(NST16):
                tot = (reps * N16_IT - q + NST16 - 1) // NST16
                if tot > 0:
                    sync.wait_ge(osem16[q], 16 * tot)

        @blk.gpsimd
        def _(gpsimd):
            gpsimd.wait_ge(gidx, 16)
            for it in range(reps):
                for cc in range(NCHT):
                    if it > 0:
                        # WAR: PE must be done reading emb of iter it-1
                        gpsimd.wait_ge(ctdone, (it - 1) * CT_IT + (cc + 1) * KD)
                    gpsimd.indirect_dma_start(
                        out=emb_sb[:, cc * D:(cc + 1) * D],
                        out_offset=None,
                        in_=table_d[:],
                        in_offset=bass.IndirectOffsetOnAxis(
                            ap=idx_sb[:, cc:cc + 1], axis=0),
                    ).then_inc(gsem[cc], 16)

        @blk.tensor
        def _(tensor):
            for w in range(WARM):
                tensor.matmul(ctps[w % 2][:, 0:P], lhsT=emb_sb[:, 0:P],
                              rhs=emb_sb[:, 0:P], start=True, stop=True)
            tensor.wait_ge(csem16, 16)
            for it in range(reps):
                def prefix(cc):
                    tensor.wait_ge(gsem[cc], 16 * (it + 1))
                    jc = it * NCHT + cc
                    if jc >= 2:
                        # WAR on ctps bank: chunk jc-2's NT copy and carry
                        # (carry follows NT on DVE, so one wait covers both)
                        tensor.wait_ge(carrysem, jc - 1)
                    for k in range(KD):
                        tensor.matmul(
                            ctps[jc % 2][:, k * P:(k + 1) * P],
                            lhsT=emb_sb[:, cc * D + k * P: cc * D + (k + 1) * P],
                            rhs=utw_ap(cc % NCHUNK),
                            start=True, stop=True).then_inc(ctdone, 1)

                def gemm_block(g, mc, bi):
                    if g == 0:
                        tensor.wait_ge(ctsbD, it * NCHT + mc + 1)
                    for nin in range(VGRP):
                        a = it * GM_IT + bi * VGRP + nin
                        q = a // 2
                        if q >= 3:
                            qe = q - 3
                            sem = asem if _pair_engine(qe) == "a" else dsem
                            tensor.wait_ge(sem, _pair_count(qe))
                        gq, sl = gps[(a // 2) % 3], a % 2
                        if it == 0 and mc == (0 if is16(mc) else 1) and nin == 0:
                            if is16(mc):
                                tensor.wait_ge(wsem16[g], 96)
                            else:
                                tensor.wait_ge(w8sem[g], 48)
                                if g == 0:
                                    tensor.wait_ge(k3sem, 1)
                        if is16(mc):
                            bt = mc // NCHUNK
                            for k in range(KD):
                                mm = tensor.matmul(
                                    gq[:, sl:sl + 1, 0:NV],
                                    lhsT=ct16[:, k:k + 1, bt * P:(bt + 1) * P],
                                    rhs=wt_sb[k][:, g * GCOLS + nin * NV:
                                                 g * GCOLS + (nin + 1) * NV],
                                    start=(k == 0), stop=(k == KD - 1))
                        else:
                            tensor.matmul(
                                gq[:, sl:sl + 1, 0:NV],
                                lhsT=ct8[:, 0:2, mc * P:(mc + 1) * P],
                                rhs=wt8_sb[:, g * KD:g * KD + 2,
                                           nin * NV:(nin + 1) * NV],
                                start=True, stop=False, perf_mode=DR)
                            # rhs k2 plane repeated via stride-0 broadcast;
                            # the lhsT k3 plane is real zeros (DVE memset)
                            mm = tensor.matmul(
                                gq[:, sl:sl + 1, 0:NV],
                                lhsT=ct8[:, 2:4, mc * P:(mc + 1) * P],
                                rhs=wt8_sb[:, g * KD + 2:g * KD + 3,
                                           nin * NV:(nin + 1) * NV]
                                .broadcast_to([P, 2, NV]),
                                start=False, stop=True, perf_mode=DR)
                        mm.then_inc(pegemm, 1)

                # prefixes run in NATURAL chunk order (the carry chain),
                # two positions ahead of the block sweep
                prefix(0)
                prefix(1)
                for bi, (g, mc) in enumerate(block_seq()):
                    if g == 0:
                        j = bi // NGRP
                        if j + 2 < NCHT:
                            prefix(j + 2)
                    gemm_block(g, mc, bi)

        def make_evict(engine, mulop, mysem):
            def evict(it, bi, g, mc, pi):
                buf, sem, gi, nst = stg_buf(it, bi)
                if gi >= nst:
                    engine.wait_ge(sem, 16 * (gi // nst))
                a1 = it * GM_IT + bi * VGRP + pi * 2 + 2
                engine.wait_ge(pegemm, a1)
                a0 = it * GM_IT + bi * VGRP + pi * 2
                mulop(buf[:, pi * 2:(pi + 1) * 2, :],
                      gps[(a0 // 2) % 3][:, 0:2, 0:NV],
                      evc_ap(mc % NCHUNK)).then_inc(mysem, 1)
            return evict

        @blk.scalar
        def _(scalar):
            scalar.wait_ge(csem32, 16)
            evict = make_evict(scalar, scalar.mul, asem)
            for it in range(reps):
                for bi, (g, mc) in enumerate(block_seq()):
                    for pi in range(2):
                        if _pair_engine(2 * bi + pi) == "a":
                            evict(it, bi, g, mc, pi)

        @blk.vector
        def _(vector):
            # zero the padded 4th k-slice of NT: garbage fp8 here could be
            # NaN and poison the DoubleRow accumulation (the matching rhs
            # plane is a stride-0 repeat of real data, so lhsT zeros rule)
            vector.memset(ct8[:, 3:4, :], 0).then_inc(k3sem, 1)
            vector.wait_ge(csem32, 16)
            evict = make_evict(vector, vector.tensor_scalar_mul, dsem)
            for it in range(reps):
                def ncss(cc):
                    """carrysc ops issued through chunk cc's fused NT."""
                    return sum(1 for x in range(cc + 1) if x % NCHUNK != 0)

                def ntD(cc):
                    """Fused NT copy + carry update + next carrysc, on DVE.

                    Same-engine RAW hazards (engine writeback overlaps the
                    next op's operand fetch) are fenced with self-waits on
                    the producing op's semaphore -- the sem fires only after
                    the write retires.
                    """
                    jc = it * NCHT + cc
                    c = cc % NCHUNK
                    if it > 0 and cc == 0:
                        # WAR: gemm of iter it-1 must be done reading NT
                        vector.wait_ge(pegemm, it * GM_IT)
                    vector.wait_ge(ctdone, jc * KD + KD)
                    src = ctps[jc % 2][:].rearrange("p (k t) -> p k t", k=KD)
                    if c == 0:
                        bt = cc // NCHUNK
                        vector.tensor_scalar_mul(
                            ct16[:, :, bt * P:(bt + 1) * P],
                            src, 1.0).then_inc(ctsbD, 1)
                    else:
                        # RAW fence: carrysc(cc) write must have retired
                        vector.wait_ge(cssem, it * (NCHT - NB) + ncss(cc))
                        cb = carrysc_sb[:, cc * KD:(cc + 1) * KD] \
                            .unsqueeze(-1).broadcast_to([P, KD, P])
                        vector.scalar_tensor_tensor(
                            ct8[:, 0:KD, cc * P:(cc + 1) * P], src,
                            float(2.0 ** -E_NT[c]), cb,
                            Alu.mult, Alu.add).then_inc(ctsbD, 1)
                    lastc = ctps[jc % 2][:, P - 1:KD * P:P]
                    dst = carry_sb[:, cc * KD:(cc + 1) * KD]
                    if c == 0:
                        vector.tensor_scalar_mul(dst, lastc, 1.0
                                                 ).then_inc(carrysem, 1)
                    else:
                        # RAW fence: carry(cc-1) write must have retired
                        vector.wait_ge(carrysem, jc)
                        vector.tensor_tensor(
                            dst, lastc,
                            carry_sb[:, (cc - 1) * KD:cc * KD],
                            Alu.add).then_inc(carrysem, 1)
                    # pre-scaled carry for the NEXT chunk's fused NT
                    nxt = cc + 1
                    if nxt < NCHT and nxt % NCHUNK != 0:
                        # RAW fence: carry(cc) write must have retired
                        vector.wait_ge(carrysem, jc + 1)
                        vector.tensor_scalar_mul(
                            carrysc_sb[:, nxt * KD:(nxt + 1) * KD],
                            carry_sb[:, cc * KD:(cc + 1) * KD],
                            float(2.0 ** -E_NT[nxt % NCHUNK])
                        ).then_inc(cssem, 1)

                # NT chain in NATURAL chunk order, one position ahead
                ntD(0)
                ntD(1)
                for bi, (g, mc) in enumerate(block_seq()):
                    if g == 0:
                        j = bi // NGRP
                        if j + 2 < NCHT:
                            ntD(j + 2)
                    for pi in range(2):
                        if _pair_engine(2 * bi + pi) == "d":
                            evict(it, bi, g, mc, pi)

    return nc


@functools.lru_cache(maxsize=None)
def _get_program(has_bias: bool, reps: int = 1, dbg: bool = False):
    return _build(has_bias, reps, dbg)


@functools.lru_cache(maxsize=None)
def _host_consts():
    import ml_dtypes
    t = np.arange(T, dtype=np.float64)
    den = (t + 1.0) * (t + 2.0) / 2.0
    s = np.arange(P)
    tril_t = (s[:, None] <= s[None, :]).astype(np.float32)  # [s, t] s<=t
    c16 = np.zeros((P, NCHUNK * P), dtype=ml_dtypes.bfloat16)
    for c in range(NCHUNK):
        posw = (np.arange(c * P, (c + 1) * P, dtype=np.float32) + 1.0)
        c16[:, c * P:(c + 1) * P] = (posw[:, None] * tril_t
                                     ).astype(ml_dtypes.bfloat16)
    c32 = np.zeros((P, NCHUNK), dtype=np.float32)
    for c in range(NCHUNK):
        sc = 1.0 if c == 0 else 2.0 ** (E_NT[c] - EW + S_OUT[c])
        c32[:, c] = (sc / den[c * P:(c + 1) * P]).astype(np.float32)
    return c16, c32


def make_in_maps(context, emb_table, W, b):
    import ml_dtypes
    context = np.asarray(context)
    emb_table = np.asarray(emb_table, dtype=np.float32)
    W = np.asarray(W, dtype=np.float32)
    b = np.asarray(b, dtype=np.float32)
    has_bias = bool(np.any(b))

    table16 = np.ascontiguousarray(emb_table.astype(ml_dtypes.bfloat16))
    wt_full = np.ascontiguousarray(W.T.astype(ml_dtypes.bfloat16))   # (D, V)
    w8_full = (W.T.astype(np.float32) * (2.0 ** EW)).astype(
        ml_dtypes.float8_e4m3)                                       # (D, V)
    c16, c32 = _host_consts()

    in_maps = []
    for ci in range(NCORE):
        vg, bg = ci % NVG, ci // NVG
        idx = np.concatenate(
            [context[bg * NB + bt].reshape(NCHUNK, P).T for bt in range(NB)],
            axis=1).astype(np.int32)           # [p, cc]
        wt = np.ascontiguousarray(wt_full[:, vg * V_CORE:(vg + 1) * V_CORE])
        w8 = np.zeros((P, NGRP * KD, GCOLS), dtype=ml_dtypes.float8_e4m3)
        for g in range(NGRP):
            for k in range(KD):
                w8[:, g * KD + k, :] = w8_full[
                    k * P:(k + 1) * P,
                    vg * V_CORE + g * GCOLS:vg * V_CORE + (g + 1) * GCOLS]
        in_maps.append({"idx": np.ascontiguousarray(idx), "table": table16,
                        "wt": wt, "wt8": w8.reshape(P, NGRP * KD * GCOLS),
                        "cst16": c16, "cst32": c32})
    return in_maps, has_bias


def kernel(context, emb_table, W, b):
    in_maps, has_bias = make_in_maps(context, emb_table, W, b)
    nc = _get_program(False)
    res = None
    for attempt in range(3):
        try:
            res = run_bass_kernel_spmd(nc, in_maps, list(range(NCORE)))
            break
        except Exception:
            # transient NRT errors on the axon-tunneled device: retry
            if attempt == 2:
                raise
            import time
            time.sleep(10.0 * (attempt + 1))

    deq = np.array([2.0 ** -S_OUT[c] for c in range(1, NCHUNK)],
                   dtype=np.float32)[None, :, None, None]
    out = np.empty((B, T, V), dtype=np.float32)
    for ci in range(NCORE):
        vg, bg = ci % NVG, ci // NVG
        vsl = slice(vg * V_CORE, (vg + 1) * V_CORE)
        o16 = np.asarray(res.results[ci]["out16"]).astype(np.float32)
        o8 = (np.asarray(res.results[ci]["out8"]).astype(np.float32)
              .reshape(NB, NCHUNK - 1, P, V_CORE) * deq)
        for bt in range(NB):
            bfull = bg * NB + bt
            out[bfull, 0:P, vsl] = o16[bt * P:(bt + 1) * P]
            out[bfull, P:T, vsl] = o8[bt].reshape((NCHUNK - 1) * P, V_CORE)
    if has_bias:
        out += np.asarray(b, dtype=np.float32)[None, None, :]
    return out


# revision 36
# speedup vs baseline: 1.0302x; 1.0302x over previous
"""BagOfWords Trainium2 kernel (fp8 DoubleRow pipeline).

Reference computation (per batch b):
    emb    = emb_table[context]                      # (T, D) gather
    logits = emb @ W.T + b                           # (T, V)
    out[t] = (sum_{s<=t} (s+1) * logits[s]) / den[t] # weighted causal cum-avg
    den[t] = (t+1)(t+2)/2

Key identity: the weighted cumsum commutes with the GEMM:
    out[t, v] = (num[t] @ W[v]) / den[t] + b[v]
    num[t, d] = sum_{s<=t} (s+1) * emb[s, d]
so the O(T*V) cumsum collapses onto the tiny (T, D) embedding side.
Per 128-token chunk (PE / DVE):
    psum[d, t] = sum_s emb[s, d] * UTW_c[s, t]       # prefix matmul per d-chunk
    NT[d, t]   = psum[d, t] + carry_prev[d]          # DVE copy w/ carry scalar
with the carry chain kept exact in fp32 (carry_sb) via paired DVE
tensor_tensor updates of the psum's last columns.

fp8 acceleration: the big GEMM out = NT.T @ W.T runs in fp8e4m3 with
MatmulPerfMode.DoubleRow -- K=256 per matmul at 0.5 cycles/moving-column,
2x the bf16 rate.  D=384 is zero-padded to 512 (2 DoubleRow matmuls/tile;
the 4th k-slice of both NT and W is zeroed).  Precision: the output's
global max lives at EARLY tokens (den[t] ~ t^2 makes late outputs tiny),
so chunk 0 of each batch stays on the bf16 path (bf16 NT, bf16 W, bf16
output) while chunks 1..7 use fp8 NT (per-chunk pow2 scale 2^-E_NT[c]),
fp8 W (2^EW), and fp8 *output* (per-chunk 2^S_OUT[c] folded into the
eviction constant, dequantized on host).  Measured end-to-end rel err
~5e-3 vs the fp32 reference (gate is 2e-2).

fp8 output also cuts the dominant HBM store traffic 2x (16.4 -> 9.2
MB/core total DMA ~16 MB ~ 44 us at 360 GB/s), and DoubleRow cuts PE time
80 -> 36 us.  PSUM->SBUF evictions (per-partition 1/den scale + dtype
convert) are the third resource: GPSIMD cannot touch PSUM, so they are
split between ACT and DVE in 2-tile (1000-column) pairs over 2-bank PSUM
tensors to amortize init overhead; DVE additionally owns the NT copies
((psum + carry)*2^-e in one two-scalar tensor_scalar op) and the paired
carry updates.

Sharding (8 cores): 4-way over B x 2-way over V.  Each core gathers 2
batches (2048 rows) and holds half of W (bf16 + fp8 copies).

Raw Bass with manual semaphores (one wait per instruction): the walrus build
in this container rejects instructions carrying multiple sem waits.

DMA semaphore discipline: a DMA's 16 per-SDMA-engine sem increments interleave
arbitrarily with other in-flight DMAs on the same semaphore, so every
concurrently-outstanding DMA group gets its own semaphore, waited to exactly
16 per iteration.

reps>1 repeats the whole pipeline inside one NEFF (used only for timing).
"""

import functools
import os
from contextlib import ExitStack

import numpy as np

import concourse.bass as bass
from concourse import mybir
from concourse.bass_utils import run_bass_kernel_spmd

B, T, V, D = 8, 1024, 8000, 384
P = 128
NCORE = 8
NCHUNK = T // P                 # 8 token chunks per batch
KD = D // P                     # 3 real contraction chunks
NV = 500                        # vocab tile (one fp32 PSUM bank half)
VGRP = 4                        # vocab tiles per store group
F32 = mybir.dt.float32
BF16 = mybir.dt.bfloat16
F8 = mybir.dt.float8e4
DR = mybir.MatmulPerfMode.DoubleRow
Alu = mybir.AluOpType

NVG = 2                         # vocab groups (cores split 4B x 2V)
WARM = int(os.environ.get("BOW_WARM", "0"))
NB = NVG                        # batches per core
V_CORE = V // NVG               # 4000 vocab columns per core
BT = NB * T                     # 2048 tokens per core
NCHT = NB * NCHUNK              # 16 token chunks per core
NTV = V_CORE // NV              # 8 vocab tiles per core
NGRP = NTV // VGRP              # 2 store column groups
GCOLS = VGRP * NV               # 2000 columns per weight/store group
NBLK = NGRP * NCHT              # 32 gemm blocks per iteration
GM_IT = NBLK * VGRP             # gemm tiles per iteration
CT_IT = NCHT * KD               # NT copies per iteration
NPAIR = GM_IT // 2              # eviction pairs per iteration (64)
NST8 = 8                        # fp8 staging buffers
NST16 = 3                       # bf16 staging buffers

# fp8 scale exponents (host-validated: global rel err ~5.4e-3)
EW = 6                                       # W8 = W * 2^EW
E_NT = [0, 6, 6, 7, 8, 8, 8, 9]              # NT8 = NT * 2^-E_NT[c]
S_OUT = [0, 11, 11, 11, 11, 12, 12, 12]      # out8 = out * 2^S_OUT[c]

# one single-chunk gather per 128 tokens (multi-chunk offset APs scramble
# the destination layout on real hardware)
GATHER_GROUPS = [1] * NCHT

# gemm block sweep order (see block_seq): fp8 chunks first, chunk-0s after
# their batch's fp8 run has started; invariant CHUNK_ORDER[j] <= j+1
CHUNK_ORDER = [1, 2, 3, 4, 5, 6, 7, 0, 9, 8, 10, 11, 12, 13, 14, 15]

# --- eviction pair -> engine assignment ---------------------------------
# Blocks sweep g-INNER ((0,mc),(1,mc),(0,mc+1),...) so each chunk's NT-copy
# work spreads over two block periods.  Block bi has pairs (2bi, 2bi+1).
# DVE owns the (fused) NT copies + carry chain (~0.79us/chunk), so ACT
# takes 5 of every 8 pairs: per 2 chunks ACT 5x1.02 = 5.09us vs DVE
# 3x1.17 + 2x0.79 = 5.08us.

_PAT = ["a", "d", "a", "a", "d", "a", "a", "d"]


def _pair_engine(q):
    return _PAT[q % len(_PAT)]

_A_IT = sum(1 for x in range(NPAIR) if _pair_engine(x) == "a")
_D_IT = NPAIR - _A_IT


def _pair_count(q):
    """1-based per-engine count of pair q among pairs of its engine."""
    e = _pair_engine(q)
    it, qq = divmod(q, NPAIR)
    base = (_A_IT if e == "a" else _D_IT) * it
    return base + sum(1 for x in range(qq + 1) if _pair_engine(x) == e)


def _block_evict_counts(it, bi):
    """Cumulative (asem, dsem) counts once block bi's pairs are evicted."""
    a = sum(1 for x in range(2 * bi + 2) if _pair_engine(x) == "a") + _A_IT * it
    d = sum(1 for x in range(2 * bi + 2) if _pair_engine(x) == "d") + _D_IT * it
    return a, d


def _build(has_bias: bool, reps: int = 1, dbg: bool = False):
    nc = bass.Bass("TRN2", target_bir_lowering=False, debug=False)

    idx_d = nc.dram_tensor("idx", [P, NCHT], mybir.dt.int32, kind="ExternalInput")
    table_d = nc.dram_tensor("table", [V, D], BF16, kind="ExternalInput")
    wt_d = nc.dram_tensor("wt", [D, V_CORE], BF16, kind="ExternalInput")
    wt8_d = nc.dram_tensor("wt8", [P, NGRP * KD * GCOLS], F8, kind="ExternalInput")
    cst16_d = nc.dram_tensor("cst16", [P, NCHUNK * P], BF16, kind="ExternalInput")
    cst32_d = nc.dram_tensor("cst32", [P, NCHUNK], F32, kind="ExternalInput")
    out16_d = nc.dram_tensor("out16", [NB * P, V_CORE], BF16, kind="ExternalOutput")
    out8_d = nc.dram_tensor("out8", [NB * (NCHUNK - 1) * P, V_CORE], F8,
                            kind="ExternalOutput")

    with ExitStack() as ctx:
        e = ctx.enter_context
        # SBUF
        idx_sb = e(nc.sbuf_tensor("idx_sb", [P, NCHT], mybir.dt.int32))
        cst16 = e(nc.sbuf_tensor("cst16_sb", [P, NCHUNK * P], BF16))
        cst32 = e(nc.sbuf_tensor("cst32_sb", [P, NCHUNK], F32))
        emb_sb = e(nc.sbuf_tensor("emb_sb", [P, NCHT * D], BF16))
        # bf16 NT: chunk 0 of each batch only
        ct16 = e(nc.sbuf_tensor("ct16", [P, KD, NB * P], BF16))
        # fp8 NT: [p, k-slice (4th zeroed), token]; chunk-0 columns unused
        ct8 = e(nc.sbuf_tensor("ct8", [P, 4, BT], F8))
        carry_sb = e(nc.sbuf_tensor("carry_sb", [P, KD * NCHT], F32))
        # pre-scaled carry columns (carry * 2^-e) for the fused NT copy
        carrysc_sb = e(nc.sbuf_tensor("carrysc_sb", [P, KD * NCHT], F32))
        wt_sb = [e(nc.sbuf_tensor(f"wt{k}", [P, V_CORE], BF16)) for k in range(KD)]
        wt8_sb = e(nc.sbuf_tensor("wt8_sb", [P, NGRP * KD, GCOLS], F8))
        ostg8 = [e(nc.sbuf_tensor(f"ostg8_{q}", [P, VGRP, NV], F8))
                 for q in range(NST8)]
        ostg16 = [e(nc.sbuf_tensor(f"ostg16_{q}", [P, VGRP, NV], BF16))
                  for q in range(NST16)]
        # PSUM: 3 x 2-bank gemm pair tensors + 2 x 1-bank prefix tensors
        gps = [e(nc.psum_tensor(f"gps{i}", [P, 2, 512], F32)) for i in range(3)]
        ctps = [e(nc.psum_tensor(f"ctps{i}", [P, KD * P], F32)) for i in range(2)]
        # sems
        gidx = e(nc.semaphore("gidx"))
        csem16 = e(nc.semaphore("csem16"))
        csem32 = e(nc.semaphore("csem32"))
        wsem16 = [e(nc.semaphore(f"wsem16_{g}")) for g in range(NGRP)]
        w8sem = [e(nc.semaphore(f"w8sem_{g}")) for g in range(NGRP)]
        k3sem = e(nc.semaphore("k3sem"))
        gsem = [e(nc.semaphore(f"gsem{gg}")) for gg in range(NCHT)]
        ctdone = e(nc.semaphore("ctdone"))      # prefix psum matmuls (PE)
        ctsbD = e(nc.semaphore("ctsbD"))        # fused NT copies (DVE)
        carrysem = e(nc.semaphore("carrysem"))  # carry updates (DVE)
        cssem = e(nc.semaphore("cssem"))        # scaled carry cols (DVE)
        pegemm = e(nc.semaphore("pegemm"))      # gemm tiles (PE)
        asem = e(nc.semaphore("asem"))          # ACT pair evictions
        dsem = e(nc.semaphore("dsem"))          # DVE pair evictions
        osem8 = [e(nc.semaphore(f"osem8_{q}")) for q in range(NST8)]
        osem16 = [e(nc.semaphore(f"osem16_{q}")) for q in range(NST16)]
        blk = e(nc.Block())

        utw_ap = lambda c: cst16[:, c * P:(c + 1) * P]
        evc_ap = lambda c: cst32[:, c:c + 1]

        # block order = gemm order: g-INNER ((0,mc),(1,mc),(0,mc'),...) over
        # CHUNK_ORDER, which runs the fp8 chunks FIRST (their weights are
        # 1.5 MB vs bf16's 3 MB, so the gemm starts ~15us earlier) and slots
        # each batch's bf16 chunk-0 in once wt16 has landed.  Legal because
        # the NT chain (natural order 0..15) is decoupled from block order:
        # CHUNK_ORDER[j] <= j+1 for all j.
        def block_seq():
            for mc in CHUNK_ORDER:
                for g in range(NGRP):
                    yield g, mc

        def is16(mc):
            return mc % NCHUNK == 0

        # staging rotation indices, per iteration-local block index
        _stg = {}
        n8 = n16 = 0
        for _bi, (_g, _mc) in enumerate(block_seq()):
            if is16(_mc):
                _stg[_bi] = ("16", n16)
                n16 += 1
            else:
                _stg[_bi] = ("8", n8)
                n8 += 1
        N8_IT, N16_IT = n8, n16      # 28 fp8 / 4 bf16 blocks per iter

        def stg_buf(it, bi):
            kind, n = _stg[bi]
            if kind == "8":
                gi = it * N8_IT + n
                return ostg8[gi % NST8], osem8[gi % NST8], gi, NST8
            gi = it * N16_IT + n
            return ostg16[gi % NST16], osem16[gi % NST16], gi, NST16

        @blk.sync
        def _(sync):
            # idx first: the gather chain is the critical path
            sync.dma_start(idx_sb[:], idx_d[:]).then_inc(gidx, 16)
            sync.dma_start(cst32[:], cst32_d[:]).then_inc(csem32, 16)
            sync.dma_start(cst16[:], cst16_d[:]).then_inc(csem16, 16)
            # weights in need order (fp8 blocks run first), in ~0.7us bands
            # so the gather transfers interleave on the DMA bus
            for g in range(NGRP):
                for k in range(KD):
                    sync.dma_start(
                        wt8_sb[:, g * KD + k:g * KD + k + 1, :],
                        wt8_d[:, (g * KD + k) * GCOLS:(g * KD + k + 1) * GCOLS]
                    ).then_inc(w8sem[g], 16)
            for g in range(NGRP):
                for k in range(KD):
                    for h in range(2):
                        c0 = g * GCOLS + h * (GCOLS // 2)
                        sync.dma_start(
                            wt_sb[k][:, c0:c0 + GCOLS // 2],
                            wt_d[k * P:(k + 1) * P, c0:c0 + GCOLS // 2]
                        ).then_inc(wsem16[g], 16)
            # output stores in gemm-block order
            for it in range(reps):
                for bi, (g, mc) in enumerate(block_seq()):
                    buf, sem, gi, nst = stg_buf(it, bi)
                    ac, dc = _block_evict_counts(it, bi)
                    sync.wait_ge(asem, ac)
                    if dc > 0:
                        sync.wait_ge(dsem, dc)
                    if is16(mc):
                        bt = mc // NCHUNK
                        sync.dma_start(
                            out16_d[bt * P:(bt + 1) * P,
                                    g * GCOLS:(g + 1) * GCOLS],
                            buf[:, :, :]).then_inc(sem, 16)
                    else:
                        bt, c = divmod(mc, NCHUNK)
                        f = bt * (NCHUNK - 1) + (c - 1)
                        sync.dma_start(
                            out8_d[f * P:(f + 1) * P,
                                   g * GCOLS:(g + 1) * GCOLS],
                            buf[:, :, :]).then_inc(sem, 16)
            for q in range(NST8):
                tot = (reps * N8_IT - q + NST8 - 1) // NST8
                if tot > 0:
                    sync.wait_ge(osem8[q], 16 * tot)
            for q in range(NST16):
                tot = (reps * N16_IT - q + NST16 - 1) // NST16
                if tot > 0:
                    sync.wait_ge(osem16[q], 16 * tot)

        @blk.gpsimd
        def _(gpsimd):
            gpsimd.wait_ge(gidx, 16)
            for it in range(reps):
                for cc in range(NCHT):
                    if it > 0:
                        # WAR: PE must be done reading emb of iter it-1
                        gpsimd.wait_ge(ctdone, (it - 1) * CT_IT + (cc + 1) * KD)
                    gpsimd.indirect_dma_start(
                        out=emb_sb[:, cc * D:(cc + 1) * D],
                        out_offset=None,
                        in_=table_d[:],
                        in_offset=bass.IndirectOffsetOnAxis(
                            ap=idx_sb[:, cc:cc + 1], axis=0),
                    ).then_inc(gsem[cc], 16)

        @blk.tensor
        def _(tensor):
            for w in range(WARM):
                tensor.matmul(ctps[w % 2][:, 0:P], lhsT=emb_sb[:, 0:P],
                              rhs=emb_sb[:, 0:P], start=True, stop=True)
            tensor.wait_ge(csem16, 16)
            for it in range(reps):
                def prefix(cc):
                    tensor.wait_ge(gsem[cc], 16 * (it + 1))
                    jc = it * NCHT + cc
                    if jc >= 2:
                        # WAR on ctps bank: chunk jc-2's NT copy and carry
                        # (carry follows NT on DVE, so one wait covers both)
                        tensor.wait_ge(carrysem, jc - 1)
                    for k in range(KD):
                        tensor.matmul(
                            ctps[jc % 2][:, k * P:(k + 1) * P],
                            lhsT=emb_sb[:, cc * D + k * P: cc * D + (k + 1) * P],
                            rhs=utw_ap(cc % NCHUNK),
                            start=True, stop=True).then_inc(ctdone, 1)

                def gemm_block(g, mc, bi):
                    if g == 0:
                        tensor.wait_ge(ctsbD, it * NCHT + mc + 1)
                    for nin in range(VGRP):
                        a = it * GM_IT + bi * VGRP + nin
                        q = a // 2
                        if q >= 3:
                            qe = q - 3
                            sem = asem if _pair_engine(qe) == "a" else dsem
                            tensor.wait_ge(sem, _pair_count(qe))
                        gq, sl = gps[(a // 2) % 3], a % 2
                        if it == 0 and mc == (0 if is16(mc) else 1) and nin == 0:
                            if is16(mc):
                                tensor.wait_ge(wsem16[g], 96)
                            else:
                                tensor.wait_ge(w8sem[g], 48)
                                if g == 0:
                                    tensor.wait_ge(k3sem, 1)
                        if is16(mc):
                            bt = mc // NCHUNK
                            for k in range(KD):
                                mm = tensor.matmul(
                                    gq[:, sl:sl + 1, 0:NV],
                                    lhsT=ct16[:, k:k + 1, bt * P:(bt + 1) * P],
                                    rhs=wt_sb[k][:, g * GCOLS + nin * NV:
                                                 g * GCOLS + (nin + 1) * NV],
                                    start=(k == 0), stop=(k == KD - 1))
                        else:
                            tensor.matmul(
                                gq[:, sl:sl + 1, 0:NV],
                                lhsT=ct8[:, 0:2, mc * P:(mc + 1) * P],
                                rhs=wt8_sb[:, g * KD:g * KD + 2,
                                           nin * NV:(nin + 1) * NV],
                                start=True, stop=False, perf_mode=DR)
                            # rhs k2 plane repeated via stride-0 broadcast;
                            # the lhsT k3 plane is real zeros (DVE memset)
                            mm = tensor.matmul(
                                gq[:, sl:sl + 1, 0:NV],
                                lhsT=ct8[:, 2:4, mc * P:(mc + 1) * P],
                                rhs=wt8_sb[:, g * KD + 2:g * KD + 3,
                                           nin * NV:(nin + 1) * NV]
                                .broadcast_to([P, 2, NV]),
                                start=False, stop=True, perf_mode=DR)
                        mm.then_inc(pegemm, 1)

                # prefixes run in NATURAL chunk order (the carry chain),
                # two positions ahead of the block sweep
                prefix(0)
                prefix(1)
                for bi, (g, mc) in enumerate(block_seq()):
                    if g == 0:
                        j = bi // NGRP
                        if j + 2 < NCHT:
                            prefix(j + 2)
                    gemm_block(g, mc, bi)

        def make_evict(engine, mulop, mysem):
            def evict(it, bi, g, mc, pi):
                buf, sem, gi, nst = stg_buf(it, bi)
                if gi >= nst:
                    engine.wait_ge(sem, 16 * (gi // nst))
                a1 = it * GM_IT + bi * VGRP + pi * 2 + 2
                engine.wait_ge(pegemm, a1)
                a0 = it * GM_IT + bi * VGRP + pi * 2
                mulop(buf[:, pi * 2:(pi + 1) * 2, :],
                      gps[(a0 // 2) % 3][:, 0:2, 0:NV],
                      evc_ap(mc % NCHUNK)).then_inc(mysem, 1)
            return evict

        @blk.scalar
        def _(scalar):
            scalar.wait_ge(csem32, 16)
            evict = make_evict(scalar, scalar.mul, asem)
            for it in range(reps):
                for bi, (g, mc) in enumerate(block_seq()):
                    for pi in range(2):
                        if _pair_engine(2 * bi + pi) == "a":
                            evict(it, bi, g, mc, pi)

        @blk.vector
        def _(vector):
            # zero the padded 4th k-slice of NT: garbage fp8 here could be
            # NaN and poison the DoubleRow accumulation (the matching rhs
            # plane is a stride-0 repeat of real data, so lhsT zeros rule)
            vector.memset(ct8[:, 3:4, :], 0).then_inc(k3sem, 1)
            vector.wait_ge(csem32, 16)
            evict = make_evict(vector, vector.tensor_scalar_mul, dsem)
            for it in range(reps):
                def ncss(cc):
                    """carrysc ops issued through chunk cc's fused NT."""
                    return sum(1 for x in range(cc + 1) if x % NCHUNK != 0)

                # Same-engine RAW hazards (engine writeback overlaps the next
                # op's operand fetch) are fenced with self-waits on the
                # producing op's semaphore.  The producers are scheduled at
                # the previous position, behind >=1us of evictions, so the
                # fences are satisfied on arrival in steady state.

                def ntD(cc):
                    """Fused NT copy + carry update for chunk cc, on DVE."""
                    jc = it * NCHT + cc
                    c = cc % NCHUNK
                    if it > 0 and cc == 0:
                        # WAR: gemm of iter it-1 must be done reading NT
                        vector.wait_ge(pegemm, it * GM_IT)
                    vector.wait_ge(ctdone, jc * KD + KD)
                    src = ctps[jc % 2][:].rearrange("p (k t) -> p k t", k=KD)
                    if c == 0:
                        bt = cc // NCHUNK
                        vector.tensor_scalar_mul(
                            ct16[:, :, bt * P:(bt + 1) * P],
                            src, 1.0).then_inc(ctsbD, 1)
                    else:
                        # RAW fence: carrysc(cc) write retired
                        vector.wait_ge(cssem, it * (NCHT - NB) + ncss(cc))
                        cb = carrysc_sb[:, cc * KD:(cc + 1) * KD] \
                            .unsqueeze(-1).broadcast_to([P, KD, P])
                        vector.scalar_tensor_tensor(
                            ct8[:, 0:KD, cc * P:(cc + 1) * P], src,
                            float(2.0 ** -E_NT[c]), cb,
                            Alu.mult, Alu.add).then_inc(ctsbD, 1)
                    lastc = ctps[jc % 2][:, P - 1:KD * P:P]
                    dst = carry_sb[:, cc * KD:(cc + 1) * KD]
                    if c == 0:
                        vector.tensor_scalar_mul(dst, lastc, 1.0
                                                 ).then_inc(carrysem, 1)
                    else:
                        # RAW fence: carry(cc-1) write retired
                        vector.wait_ge(carrysem, jc)
                        vector.tensor_tensor(
                            dst, lastc,
                            carry_sb[:, (cc - 1) * KD:cc * KD],
                            Alu.add).then_inc(carrysem, 1)

                def csc(cc):
                    """Pre-scaled carry for chunk cc's fused NT, on DVE."""
                    if cc >= NCHT or cc % NCHUNK == 0:
                        return
                    # RAW fence: carry(cc-1) write retired
                    vector.wait_ge(carrysem, it * NCHT + cc)
                    vector.tensor_scalar_mul(
                        carrysc_sb[:, cc * KD:(cc + 1) * KD],
                        carry_sb[:, (cc - 1) * KD:cc * KD],
                        float(2.0 ** -E_NT[cc % NCHUNK])).then_inc(cssem, 1)

                # NT chain in NATURAL chunk order, one position ahead; each
                # csc producer runs a position before its consumer, behind
                # this position's evictions
                ntD(0)
                csc(1)
                ntD(1)
                csc(2)
                for bi, (g, mc) in enumerate(block_seq()):
                    if g == 0:
                        j = bi // NGRP
                        if j + 2 < NCHT:
                            ntD(j + 2)
                    for pi in range(2):
                        if _pair_engine(2 * bi + pi) == "d":
                            evict(it, bi, g, mc, pi)
                    if g == NGRP - 1:
                        j = bi // NGRP
                        csc(j + 3)

    return nc


@functools.lru_cache(maxsize=None)
def _get_program(has_bias: bool, reps: int = 1, dbg: bool = False):
    return _build(has_bias, reps, dbg)


@functools.lru_cache(maxsize=None)
def _host_consts():
    import ml_dtypes
    t = np.arange(T, dtype=np.float64)
    den = (t + 1.0) * (t + 2.0) / 2.0
    s = np.arange(P)
    tril_t = (s[:, None] <= s[None, :]).astype(np.float32)  # [s, t] s<=t
    c16 = np.zeros((P, NCHUNK * P), dtype=ml_dtypes.bfloat16)
    for c in range(NCHUNK):
        posw = (np.arange(c * P, (c + 1) * P, dtype=np.float32) + 1.0)
        c16[:, c * P:(c + 1) * P] = (posw[:, None] * tril_t
                                     ).astype(ml_dtypes.bfloat16)
    c32 = np.zeros((P, NCHUNK), dtype=np.float32)
    for c in range(NCHUNK):
        sc = 1.0 if c == 0 else 2.0 ** (E_NT[c] - EW + S_OUT[c])
        c32[:, c] = (sc / den[c * P:(c + 1) * P]).astype(np.float32)
    return c16, c32


def make_in_maps(context, emb_table, W, b):
    import ml_dtypes
    context = np.asarray(context)
    emb_table = np.asarray(emb_table, dtype=np.float32)
    W = np.asarray(W, dtype=np.float32)
    b = np.asarray(b, dtype=np.float32)
    has_bias = bool(np.any(b))

    table16 = np.ascontiguousarray(emb_table.astype(ml_dtypes.bfloat16))
    wt_full = np.ascontiguousarray(W.T.astype(ml_dtypes.bfloat16))   # (D, V)
    w8_full = (W.T.astype(np.float32) * (2.0 ** EW)).astype(
        ml_dtypes.float8_e4m3)                                       # (D, V)
    c16, c32 = _host_consts()

    in_maps = []
    for ci in range(NCORE):
        vg, bg = ci % NVG, ci // NVG
        idx = np.concatenate(
            [context[bg * NB + bt].reshape(NCHUNK, P).T for bt in range(NB)],
            axis=1).astype(np.int32)           # [p, cc]
        wt = np.ascontiguousarray(wt_full[:, vg * V_CORE:(vg + 1) * V_CORE])
        w8 = np.zeros((P, NGRP * KD, GCOLS), dtype=ml_dtypes.float8_e4m3)
        for g in range(NGRP):
            for k in range(KD):
                w8[:, g * KD + k, :] = w8_full[
                    k * P:(k + 1) * P,
                    vg * V_CORE + g * GCOLS:vg * V_CORE + (g + 1) * GCOLS]
        in_maps.append({"idx": np.ascontiguousarray(idx), "table": table16,
                        "wt": wt, "wt8": w8.reshape(P, NGRP * KD * GCOLS),
                        "cst16": c16, "cst32": c32})
    return in_maps, has_bias


def kernel(context, emb_table, W, b):
    in_maps, has_bias = make_in_maps(context, emb_table, W, b)
    nc = _get_program(False)
    res = None
    for attempt in range(3):
        try:
            res = run_bass_kernel_spmd(nc, in_maps, list(range(NCORE)))
            break
        except Exception:
            # transient NRT errors on the axon-tunneled device: retry
            if attempt == 2:
                raise
            import time
            time.sleep(10.0 * (attempt + 1))

    deq = np.array([2.0 ** -S_OUT[c] for c in range(1, NCHUNK)],
                   dtype=np.float32)[None, :, None, None]
    out = np.empty((B, T, V), dtype=np.float32)
    for ci in range(NCORE):
        vg, bg = ci % NVG, ci // NVG
        vsl = slice(vg * V_CORE, (vg + 1) * V_CORE)
        o16 = np.asarray(res.results[ci]["out16"]).astype(np.float32)
        o8 = (np.asarray(res.results[ci]["out8"]).astype(np.float32)
              .reshape(NB, NCHUNK - 1, P, V_CORE) * deq)
        for bt in range(NB):
            bfull = bg * NB + bt
            out[bfull, 0:P, vsl] = o16[bt * P:(bt + 1) * P]
            out[bfull, P:T, vsl] = o8[bt].reshape((NCHUNK - 1) * P, V_CORE)
    if has_bias:
        out += np.asarray(b, dtype=np.float32)[None, None, :]
    return out


# revision 72
# speedup vs baseline: 1.0697x; 1.0383x over previous
"""BagOfWords Trainium2 kernel (fp8 DoubleRow pipeline).

Reference computation (per batch b):
    emb    = emb_table[context]                      # (T, D) gather
    logits = emb @ W.T + b                           # (T, V)
    out[t] = (sum_{s<=t} (s+1) * logits[s]) / den[t] # weighted causal cum-avg
    den[t] = (t+1)(t+2)/2

Key identity: the weighted cumsum commutes with the GEMM:
    out[t, v] = (num[t] @ W[v]) / den[t] + b[v]
    num[t, d] = sum_{s<=t} (s+1) * emb[s, d]
so the O(T*V) cumsum collapses onto the tiny (T, D) embedding side.
Per 128-token chunk (PE / DVE):
    psum[d, t] = sum_s emb[s, d] * UTW_c[s, t]       # prefix matmul per d-chunk
    NT[d, t]   = psum[d, t] + carry_prev[d]          # DVE copy w/ carry scalar
with the carry chain kept exact in fp32 (carry_sb) via paired DVE
tensor_tensor updates of the psum's last columns.

fp8 acceleration: the big GEMM out = NT.T @ W.T runs in fp8e4m3 with
MatmulPerfMode.DoubleRow -- K=256 per matmul at 0.5 cycles/moving-column,
2x the bf16 rate.  D=384 is zero-padded to 512 (2 DoubleRow matmuls/tile;
the 4th k-slice of both NT and W is zeroed).  Precision: the output's
global max lives at EARLY tokens (den[t] ~ t^2 makes late outputs tiny),
so chunk 0 of each batch stays on the bf16 path (bf16 NT, bf16 W, bf16
output) while chunks 1..7 use fp8 NT (per-chunk pow2 scale 2^-E_NT[c]),
fp8 W (2^EW), and fp8 *output* (per-chunk 2^S_OUT[c] folded into the
eviction constant, dequantized on host).  Measured end-to-end rel err
~5e-3 vs the fp32 reference (gate is 2e-2).

fp8 output also cuts the dominant HBM store traffic 2x (16.4 -> 9.2
MB/core total DMA ~16 MB ~ 44 us at 360 GB/s), and DoubleRow cuts PE time
80 -> 36 us.  PSUM->SBUF evictions (per-partition 1/den scale + dtype
convert) are the third resource: GPSIMD cannot touch PSUM, so they are
split between ACT and DVE in 2-tile (1000-column) pairs over 2-bank PSUM
tensors to amortize init overhead; DVE additionally owns the NT copies
((psum + carry)*2^-e in one two-scalar tensor_scalar op) and the paired
carry updates.

Sharding (8 cores): 4-way over B x 2-way over V.  Each core gathers 2
batches (2048 rows) and holds half of W (bf16 + fp8 copies).

Raw Bass with manual semaphores (one wait per instruction): the walrus build
in this container rejects instructions carrying multiple sem waits.

DMA semaphore discipline: a DMA's 16 per-SDMA-engine sem increments interleave
arbitrarily with other in-flight DMAs on the same semaphore, so every
concurrently-outstanding DMA group gets its own semaphore, waited to exactly
16 per iteration.

reps>1 repeats the whole pipeline inside one NEFF (used only for timing).
"""

import functools
import os
from contextlib import ExitStack

import numpy as np

import concourse.bass as bass
from concourse import mybir
from concourse.bass_utils import run_bass_kernel_spmd

B, T, V, D = 8, 1024, 8000, 384
P = 128
NCORE = 8
NCHUNK = T // P                 # 8 token chunks per batch
KD = D // P                     # 3 real contraction chunks
NV = 500                        # vocab tile (one fp32 PSUM bank half)
VGRP = 4                        # vocab tiles per store group
F32 = mybir.dt.float32
BF16 = mybir.dt.bfloat16
F8 = mybir.dt.float8e4
DR = mybir.MatmulPerfMode.DoubleRow
Alu = mybir.AluOpType

NVG = 2                         # vocab groups (cores split 4B x 2V)
WARM = int(os.environ.get("BOW_WARM", "0"))
NB = NVG                        # batches per core
V_CORE = V // NVG               # 4000 vocab columns per core
BT = NB * T                     # 2048 tokens per core
NCHT = NB * NCHUNK              # 16 token chunks per core
NTV = V_CORE // NV              # 8 vocab tiles per core
NGRP = NTV // VGRP              # 2 store column groups
GCOLS = VGRP * NV               # 2000 columns per weight/store group
NBLK = NGRP * NCHT              # 32 gemm blocks per iteration
GM_IT = NBLK * VGRP             # gemm tiles per iteration
CT_IT = NCHT * KD               # NT copies per iteration
NPAIR = GM_IT // 2              # eviction pairs per iteration (64)
NST8 = 8                        # fp8 staging buffers
NST16 = 3                       # bf16 staging buffers

# fp8 scale exponents (host-validated: global rel err ~5.4e-3)
EW = 6                                       # W8 = W * 2^EW (+ Wr8 residual)
E_NT = [0, 6, 6, 7, 8, 8, 8, 9]              # NT8 = NT * 2^-E_NT[c], c>=1
E0N = 7                                      # chunk0: NT08 = (NT0/den) * 2^E0N
S_OUT = [0, 11, 11, 11, 11, 12, 12, 12]      # out8 = out * 2^S_OUT[c]
DEN_LAST = P * (P + 1) / 2.0                 # den[127]: un-normalizes carry 0

# one single-chunk gather per 128 tokens (multi-chunk offset APs scramble
# the destination layout on real hardware)
GATHER_GROUPS = [1] * NCHT

# gemm block sweep order: fp8 chunks first (their weights are small and
# load first), bf16-path chunk-0s once wr8 has landed; invariant
# CHUNK_ORDER[j] <= j+1 (the NT chain runs in natural order, one position
# ahead of the block sweep)
CHUNK_ORDER = [1, 2, 3, 4, 5, 6, 7, 0, 9, 10, 11, 8, 12, 13, 14, 15]

# --- the schedule ---------------------------------------------------------
# One iteration's op stream, shared by all engines (each walks it filtering
# its own op kinds; list order == every engine's queue order):
#   ("prefix", cc)      PE   3 prefix matmuls for chunk cc (natural order)
#   ("ntD", cc)         DVE  fused NT copy + carry update
#   ("res", cc)         DVE  chunk-0 NT residual (no-op unless cc%8==0)
#   ("csc", cc)         DVE  pre-scaled carry (no-op if cc%8==0)
#   ("pair", bi, pi)    PE tiles + ACT/DVE eviction, schedule pair index s
#
# The compensated chunk-0 blocks cost 3x PE per tile, so their pairs are
# DEFERRED and woven one-per-position into the following fp8 positions --
# otherwise the PE grinds ~5us per chunk-0 while the eviction engines
# starve.  Pair->engine assignment is a greedy balance of modeled ACT/DVE
# busy time.


def _mk_schedule():
    # prologue order matters: prefix(1) WAR-waits on carry(0) AND res(0)
    # (single prefix bank), so both must precede ntD(1) in the DVE queue
    ops = [("prefix", 0), ("ntD", 0), ("res", 0), ("csc", 1),
           ("prefix", 1), ("ntD", 1), ("csc", 2)]
    deferred = []
    for j, mc in enumerate(CHUNK_ORDER):
        if j + 2 < NCHT:
            ops.append(("prefix", j + 2))
            ops.append(("ntD", j + 2))
        own = [(2 * j, 0), (2 * j, 1), (2 * j + 1, 0), (2 * j + 1, 1)]
        if mc % NCHUNK == 0:
            deferred.extend(own)
            own = []
        for i, bp in enumerate(own):
            ops.append(("pair",) + bp)
            if i == 1 and deferred:
                ops.append(("pair",) + deferred.pop(0))
        ops.append(("res", j + 2))
        ops.append(("csc", j + 3))
    for bp in deferred:
        ops.append(("pair",) + bp)
    return ops


_OPS = _mk_schedule()
_BLOCKS = []
for _mc in CHUNK_ORDER:
    for _g in range(NGRP):
        _BLOCKS.append((_g, _mc))


def _csc_valid(cc):
    return cc < NCHT and cc % NCHUNK != 0


def _res_valid(cc):
    return cc < NCHT and cc % NCHUNK == 0


# tiles in schedule order: 2 per "pair" entry (nin = pi*2, pi*2+1)
_TILES = []
for _op in _OPS:
    if _op[0] == "pair":
        _bi, _pi = _op[1], _op[2]
        _TILES.append((_bi, _pi * 2))
        _TILES.append((_bi, _pi * 2 + 1))
NTIL = len(_TILES)



NSLOT = 7               # single-tile psum slots (7 banks + 1 prefix bank)

# eviction ops: tiles (2s, 2s+1) of entry s are evicted as ONE 1000-col op
# when their psum slots are consecutive; at the slot-ring wrap they fall
# back to two singles.  _EOPS entries: list of tile indices (len 1 or 2).
_EOPS = []
for _s in range(NTIL // 2):
    if (2 * _s) % NSLOT == NSLOT - 1:
        _EOPS.append([2 * _s])
        _EOPS.append([2 * _s + 1])
    else:
        _EOPS.append([2 * _s, 2 * _s + 1])
_TILE_EOP = {}
for _ei, _ts in enumerate(_EOPS):
    for _t in _ts:
        _TILE_EOP[_t] = _ei


# simpler: assign greedily in plain eop order with NT-chain load charged at
# the position granularity (14 fp8 ntD ~0.66 + 2 chunk-0 ~1.18 + csc ~0.13
# spread evenly across the 64 entries)
_DVE_SIDE = (14 * (0.525 + 0.13 + 0.13) + 2 * (0.525 + 0.13 + 0.525)) / len(_EOPS)
_ENG_EOP = []
_la = _ld = 0.0
for _ts in _EOPS:
    _ca_ = 1.018 if len(_ts) == 2 else 0.602
    _cd_ = 1.165 if len(_ts) == 2 else 0.646
    _ld += _DVE_SIDE
    if _la + _ca_ <= _ld + _cd_:
        _ENG_EOP.append("a")
        _la += _ca_
    else:
        _ENG_EOP.append("d")
        _ld += _cd_
_A_IT = _ENG_EOP.count("a")
_D_IT = len(_EOPS) - _A_IT
NEOP = len(_EOPS)
# per-eop 1-based per-engine count
_ECNT = []
_ca = _cd = 0
for _e in _ENG_EOP:
    if _e == "a":
        _ca += 1
        _ECNT.append(_ca)
    else:
        _cd += 1
        _ECNT.append(_cd)
# per-block: cumulative (a, d) eviction-op counts + last tile index
_BLK_DONE = {}
_BLK_LAST = {}
_sa = _sd = 0
_blk_seen = {}
for _ei, _ts in enumerate(_EOPS):
    if _ENG_EOP[_ei] == "a":
        _sa += 1
    else:
        _sd += 1
    for _t in _ts:
        _bi = _TILES[_t][0]
        _blk_seen[_bi] = _blk_seen.get(_bi, 0) + 1
        if _blk_seen[_bi] == VGRP:
            _BLK_DONE[_bi] = (_sa, _sd)
            _BLK_LAST[_bi] = _t
assert len(_BLK_DONE) == NBLK and NTIL == GM_IT
# stores flow in completion order (deferred chunk-0 pairs finish late);
# staging rotation follows the same order
_STORE_ORDER = sorted(range(NBLK), key=lambda bi: _BLK_LAST[bi])

_FIRST_OF_CHUNK = {}    # mc -> first schedule tile index touching it
_FIRST_W = {}           # ("8"|"r", g) -> first tile needing that weight set
for _t, (_bi, _nin) in enumerate(_TILES):
    _g, _mc = _BLOCKS[_bi]
    _FIRST_OF_CHUNK.setdefault(_mc, _t)
    _FIRST_W.setdefault(("r" if _mc % NCHUNK == 0 else "8", _g), _t)
_FIRST_R = min(_FIRST_W[("r", g)] for g in range(NGRP))
# eops grouped by schedule entry (entry s covers tiles 2s, 2s+1)
_ENTRY_EOPS = [[] for _ in range(NTIL // 2)]
for _ei, _ts in enumerate(_EOPS):
    _ENTRY_EOPS[_ts[0] // 2].append(_ei)
for _ts in _EOPS:
    assert len({_TILES[_t][0] for _t in _ts}) == 1


def _build(has_bias: bool, reps: int = 1, dbg: bool = False):
    nc = bass.Bass("TRN2", target_bir_lowering=False, debug=False)

    idx_d = nc.dram_tensor("idx", [P, NCHT], mybir.dt.int32, kind="ExternalInput")
    table_d = nc.dram_tensor("table", [V, D], BF16, kind="ExternalInput")
    wt8_d = nc.dram_tensor("wt8", [P, NGRP * KD * GCOLS], F8, kind="ExternalInput")
    wr8_d = nc.dram_tensor("wr8", [P, NGRP * KD * GCOLS], F8, kind="ExternalInput")
    cst16_d = nc.dram_tensor("cst16", [P, NCHUNK * P], BF16, kind="ExternalInput")
    cst32_d = nc.dram_tensor("cst32", [P, NCHUNK], F32, kind="ExternalInput")
    out16_d = nc.dram_tensor("out16", [NB * P, V_CORE], BF16, kind="ExternalOutput")
    out8_d = nc.dram_tensor("out8", [NB * (NCHUNK - 1) * P, V_CORE], F8,
                            kind="ExternalOutput")

    with ExitStack() as ctx:
        e = ctx.enter_context
        # SBUF
        idx_sb = e(nc.sbuf_tensor("idx_sb", [P, NCHT], mybir.dt.int32))
        cst16 = e(nc.sbuf_tensor("cst16_sb", [P, NCHUNK * P], BF16))
        cst32 = e(nc.sbuf_tensor("cst32_sb", [P, NCHUNK], F32))
        emb_sb = e(nc.sbuf_tensor("emb_sb", [P, NCHT * D], BF16))
        # fp8 NT: [p, k-slice (4th zeroed), token]; chunk-0 columns hold the
        # den-normalized chunk-0 NT (its residual lives in ct8r)
        ct8 = e(nc.sbuf_tensor("ct8", [P, 4, BT], F8))
        ct8r = e(nc.sbuf_tensor("ct8r", [P, 4, NB * P], F8))
        carry_sb = e(nc.sbuf_tensor("carry_sb", [P, KD * NCHT], F32))
        # pre-scaled carry columns (carry * 2^-e) for the fused NT copy
        carrysc_sb = e(nc.sbuf_tensor("carrysc_sb", [P, KD * NCHT], F32))
        wt8_sb = e(nc.sbuf_tensor("wt8_sb", [P, NGRP * KD, GCOLS], F8))
        wr8_sb = e(nc.sbuf_tensor("wr8_sb", [P, NGRP * KD, GCOLS], F8))
        ostg8 = [e(nc.sbuf_tensor(f"ostg8_{q}", [P, VGRP, NV], F8))
                 for q in range(NST8)]
        ostg16 = [e(nc.sbuf_tensor(f"ostg16_{q}", [P, VGRP, NV], BF16))
                  for q in range(NST16)]
        # PSUM: one 7-bank ring of single-tile gemm slots (depth 7 hides
        # the eviction round-trip; consecutive slots evict as 1000-col
        # pairs) + a SINGLE 1-bank prefix tensor: prefix(cc) runs two
        # positions after NT(cc-1) drained it, no ping-pong needed
        gpsall = e(nc.psum_tensor("gpsall", [P, NSLOT, 512], F32))
        ctps = e(nc.psum_tensor("ctps", [P, KD * P], F32))
        # sems
        gidx = e(nc.semaphore("gidx"))
        csem16 = e(nc.semaphore("csem16"))
        csem32 = e(nc.semaphore("csem32"))
        w8sem = [e(nc.semaphore(f"w8sem_{g}")) for g in range(NGRP)]
        wr8sem = [e(nc.semaphore(f"wr8sem_{g}")) for g in range(NGRP)]
        k3sem = e(nc.semaphore("k3sem"))
        gsem = [e(nc.semaphore(f"gsem{gg}")) for gg in range(NCHT)]
        ctdone = e(nc.semaphore("ctdone"))      # prefix psum matmuls (PE)
        ctsbD = e(nc.semaphore("ctsbD"))        # fused NT copies (DVE)
        ctrsem = e(nc.semaphore("ctrsem"))      # chunk-0 NT residuals (DVE)
        carrysem = e(nc.semaphore("carrysem"))  # carry updates (DVE)
        cssem = e(nc.semaphore("cssem"))        # scaled carry cols (DVE)
        pegemm = e(nc.semaphore("pegemm"))      # gemm tiles (PE)
        asem = e(nc.semaphore("asem"))          # ACT pair evictions
        dsem = e(nc.semaphore("dsem"))          # DVE pair evictions
        osem8 = [e(nc.semaphore(f"osem8_{q}")) for q in range(NST8)]
        osem16 = [e(nc.semaphore(f"osem16_{q}")) for q in range(NST16)]
        blk = e(nc.Block())

        utw_ap = lambda c: cst16[:, c * P:(c + 1) * P]
        evc_ap = lambda c: cst32[:, c:c + 1]

        def is16(mc):
            return mc % NCHUNK == 0

        # staging rotation indices, assigned in store/completion order
        _stg = {}
        n8 = n16 = 0
        for _bi in _STORE_ORDER:
            if is16(_BLOCKS[_bi][1]):
                _stg[_bi] = ("16", n16)
                n16 += 1
            else:
                _stg[_bi] = ("8", n8)
                n8 += 1
        N8_IT, N16_IT = n8, n16      # 28 fp8 / 4 compensated blocks per iter

        def stg_buf(it, bi):
            kind, n = _stg[bi]
            if kind == "8":
                gi = it * N8_IT + n
                return ostg8[gi % NST8], osem8[gi % NST8], gi, NST8
            gi = it * N16_IT + n
            return ostg16[gi % NST16], osem16[gi % NST16], gi, NST16

        @blk.sync
        def _(sync):
            # idx first: the gather chain is the critical path
            sync.dma_start(idx_sb[:], idx_d[:]).then_inc(gidx, 16)
            sync.dma_start(cst32[:], cst32_d[:]).then_inc(csem32, 16)
            sync.dma_start(cst16[:], cst16_d[:]).then_inc(csem16, 16)
            # weights in need order (main fp8 first, chunk-0 residual after),
            # in ~0.7us bands so the gather transfers interleave on the bus
            for g in range(NGRP):
                for k in range(KD):
                    sync.dma_start(
                        wt8_sb[:, g * KD + k:g * KD + k + 1, :],
                        wt8_d[:, (g * KD + k) * GCOLS:(g * KD + k + 1) * GCOLS]
                    ).then_inc(w8sem[g], 16)
            for g in range(NGRP):
                for k in range(KD):
                    sync.dma_start(
                        wr8_sb[:, g * KD + k:g * KD + k + 1, :],
                        wr8_d[:, (g * KD + k) * GCOLS:(g * KD + k + 1) * GCOLS]
                    ).then_inc(wr8sem[g], 16)
            # output stores in completion order
            for it in range(reps):
                for bi in _STORE_ORDER:
                    g, mc = _BLOCKS[bi]
                    buf, sem, gi, nst = stg_buf(it, bi)
                    ac, dc = _BLK_DONE[bi]
                    if ac > 0:
                        sync.wait_ge(asem, it * _A_IT + ac)
                    if dc > 0:
                        sync.wait_ge(dsem, it * _D_IT + dc)
                    if is16(mc):
                        bt = mc // NCHUNK
                        sync.dma_start(
                            out16_d[bt * P:(bt + 1) * P,
                                    g * GCOLS:(g + 1) * GCOLS],
                            buf[:, :, :]).then_inc(sem, 16)
                    else:
                        bt, c = divmod(mc, NCHUNK)
                        f = bt * (NCHUNK - 1) + (c - 1)
                        sync.dma_start(
                            out8_d[f * P:(f + 1) * P,
                                   g * GCOLS:(g + 1) * GCOLS],
                            buf[:, :, :]).then_inc(sem, 16)
            for q in range(NST8):
                tot = (reps * N8_IT - q + NST8 - 1) // NST8
                if tot > 0:
                    sync.wait_ge(osem8[q], 16 * tot)
            for q in range(NST16):
                tot = (reps * N16_IT - q + NST16 - 1) // NST16
                if tot > 0:
                    sync.wait_ge(osem16[q], 16 * tot)

        @blk.gpsimd
        def _(gpsimd):
            gpsimd.wait_ge(gidx, 16)
            for it in range(reps):
                for cc in range(NCHT):
                    if it > 0:
                        # WAR: PE must be done reading emb of iter it-1
                        gpsimd.wait_ge(ctdone, (it - 1) * CT_IT + (cc + 1) * KD)
                    gpsimd.indirect_dma_start(
                        out=emb_sb[:, cc * D:(cc + 1) * D],
                        out_offset=None,
                        in_=table_d[:],
                        in_offset=bass.IndirectOffsetOnAxis(
                            ap=idx_sb[:, cc:cc + 1], axis=0),
                    ).then_inc(gsem[cc], 16)

        @blk.tensor
        def _(tensor):
            for w in range(WARM):
                tensor.matmul(ctps[:, 0:P], lhsT=emb_sb[:, 0:P],
                              rhs=emb_sb[:, 0:P], start=True, stop=True)
            tensor.wait_ge(csem16, 16)
            for it in range(reps):
                def prefix(cc):
                    tensor.wait_ge(gsem[cc], 16 * (it + 1))
                    jc = it * NCHT + cc
                    if jc >= 1:
                        # WAR on the single ctps bank: chunk jc-1's NT copy
                        # and carry (carry follows NT on DVE, one wait covers
                        # both), plus its NT residual if it was a chunk-0
                        tensor.wait_ge(carrysem, jc)
                        if (jc - 1) % NCHUNK == 0:
                            bt2 = ((jc - 1) % NCHT) // NCHUNK
                            tensor.wait_ge(ctrsem,
                                           ((jc - 1) // NCHT) * NB + bt2 + 1)
                    for k in range(KD):
                        tensor.matmul(
                            ctps[:, k * P:(k + 1) * P],
                            lhsT=emb_sb[:, cc * D + k * P: cc * D + (k + 1) * P],
                            rhs=utw_ap(cc % NCHUNK),
                            start=True, stop=True).then_inc(ctdone, 1)

                def gemm_tile(t, bi, nin):
                    g, mc = _BLOCKS[bi]
                    if _FIRST_OF_CHUNK[mc] == t:
                        tensor.wait_ge(ctsbD, it * NCHT + mc + 1)
                        if is16(mc):
                            bt = mc // NCHUNK
                            tensor.wait_ge(ctrsem, it * NB + bt + 1)
                    if it == 0:
                        kind = "r" if is16(mc) else "8"
                        if _FIRST_W[(kind, g)] == t:
                            tensor.wait_ge(wr8sem[g] if kind == "r"
                                           else w8sem[g], 48)
                        if t == 0:
                            tensor.wait_ge(k3sem, 1)
                        if t == _FIRST_R:
                            tensor.wait_ge(k3sem, 2)
                    if t >= NSLOT:
                        ei = _TILE_EOP[t - NSLOT]
                        ee = _ENG_EOP[ei]
                        sem = asem if ee == "a" else dsem
                        tensor.wait_ge(sem, it * (_A_IT if ee == "a"
                                                  else _D_IT) + _ECNT[ei])
                    gq = gpsall[:, t % NSLOT:t % NSLOT + 1, :]

                    def dr_pair(lh, cols, rsb, start, stop):
                        # rhs k2 plane repeated via stride-0 broadcast;
                        # the lhsT k3 plane is real zeros (DVE memset)
                        tensor.matmul(
                            gq[:, :, 0:NV],
                            lhsT=lh[:, 0:2, cols],
                            rhs=rsb[:, g * KD:g * KD + 2,
                                    nin * NV:(nin + 1) * NV],
                            start=start, stop=False, perf_mode=DR)
                        return tensor.matmul(
                            gq[:, :, 0:NV],
                            lhsT=lh[:, 2:4, cols],
                            rhs=rsb[:, g * KD + 2:g * KD + 3,
                                    nin * NV:(nin + 1) * NV]
                            .broadcast_to([P, 2, NV]),
                            start=False, stop=stop, perf_mode=DR)

                    tcols = slice(mc * P, (mc + 1) * P)
                    if is16(mc):
                        # chunk 0: error-compensated fp8 --
                        # NT08@W8 + NT08@Wr8 + NTr08@W8
                        bt = mc // NCHUNK
                        rcols = slice(bt * P, (bt + 1) * P)
                        dr_pair(ct8, tcols, wt8_sb, True, False)
                        dr_pair(ct8, tcols, wr8_sb, False, False)
                        mm = dr_pair(ct8r, rcols, wt8_sb, False, True)
                    else:
                        mm = dr_pair(ct8, tcols, wt8_sb, True, True)
                    mm.then_inc(pegemm, 1)

                t = 0
                for op in _OPS:
                    if op[0] == "prefix":
                        prefix(op[1])
                    elif op[0] == "pair":
                        gemm_tile(t, op[1], op[2] * 2)
                        gemm_tile(t + 1, op[1], op[2] * 2 + 1)
                        t += 2

        def make_evict(engine, mulop, mysem):
            def evict(it, ei):
                ts = _EOPS[ei]
                bi, nin0 = _TILES[ts[0]]
                g, mc = _BLOCKS[bi]
                n = len(ts)
                buf, sem, gi, nst = stg_buf(it, bi)
                if gi >= nst:
                    engine.wait_ge(sem, 16 * (gi // nst))
                engine.wait_ge(pegemm, it * GM_IT + ts[-1] + 1)
                slot = ts[0] % NSLOT
                mulop(buf[:, nin0:nin0 + n, :],
                      gpsall[:, slot:slot + n, 0:NV],
                      evc_ap(mc % NCHUNK)).then_inc(mysem, 1)
            return evict

        @blk.scalar
        def _(scalar):
            scalar.wait_ge(csem32, 16)
            evict = make_evict(scalar, scalar.mul, asem)
            for it in range(reps):
                s = 0
                for op in _OPS:
                    if op[0] == "pair":
                        for ei in _ENTRY_EOPS[s]:
                            if _ENG_EOP[ei] == "a":
                                evict(it, ei)
                        s += 1

        @blk.vector
        def _(vector):
            # zero the padded 4th k-slices of NT: garbage fp8 here could be
            # NaN and poison the DoubleRow accumulation (the matching rhs
            # plane is a stride-0 repeat of real data, so lhsT zeros rule)
            vector.memset(ct8[:, 3:4, :], 0).then_inc(k3sem, 1)
            vector.memset(ct8r[:, 3:4, :], 0).then_inc(k3sem, 1)
            vector.wait_ge(csem32, 16)
            evict = make_evict(vector, vector.tensor_scalar_mul, dsem)
            for it in range(reps):
                def ncss(cc):
                    """carrysc ops issued through chunk cc's fused NT."""
                    return sum(1 for x in range(cc + 1) if x % NCHUNK != 0)

                # Same-engine RAW hazards (engine writeback overlaps the next
                # op's operand fetch) are fenced with self-waits on the
                # producing op's semaphore.  The producers are scheduled at
                # the previous position, behind >=1us of evictions, so the
                # fences are satisfied on arrival in steady state.

                def ntD(cc):
                    """Fused NT copy + carry update for chunk cc, on DVE."""
                    jc = it * NCHT + cc
                    c = cc % NCHUNK
                    if it > 0 and cc == 0:
                        # WAR: gemm of iter it-1 must be done reading NT
                        vector.wait_ge(pegemm, it * GM_IT)
                    vector.wait_ge(ctdone, jc * KD + KD)
                    src = ctps[:].rearrange("p (k t) -> p k t", k=KD)
                    if c == 0:
                        # psum holds den-NORMALIZED sums (chunk-0 UTW plane)
                        vector.tensor_scalar_mul(
                            ct8[:, 0:KD, cc * P:(cc + 1) * P],
                            src, float(2.0 ** E0N)).then_inc(ctsbD, 1)
                    else:
                        # RAW fence: carrysc(cc) write retired
                        vector.wait_ge(cssem, it * (NCHT - NB) + ncss(cc))
                        cb = carrysc_sb[:, cc * KD:(cc + 1) * KD] \
                            .unsqueeze(-1).broadcast_to([P, KD, P])
                        vector.scalar_tensor_tensor(
                            ct8[:, 0:KD, cc * P:(cc + 1) * P], src,
                            float(2.0 ** -E_NT[c]), cb,
                            Alu.mult, Alu.add).then_inc(ctsbD, 1)
                    lastc = ctps[:, P - 1:KD * P:P]
                    dst = carry_sb[:, cc * KD:(cc + 1) * KD]
                    if c == 0:
                        # un-normalize: raw carry = psum_last * den[last]
                        vector.tensor_scalar_mul(dst, lastc, float(DEN_LAST)
                                                 ).then_inc(carrysem, 1)
                    else:
                        # RAW fence: carry(cc-1) write retired
                        vector.wait_ge(carrysem, jc)
                        vector.tensor_tensor(
                            dst, lastc,
                            carry_sb[:, (cc - 1) * KD:cc * KD],
                            Alu.add).then_inc(carrysem, 1)

                def csc(cc):
                    """Pre-scaled carry for chunk cc's fused NT, on DVE."""
                    if cc >= NCHT or cc % NCHUNK == 0:
                        return
                    # RAW fence: carry(cc-1) write retired
                    vector.wait_ge(carrysem, it * NCHT + cc)
                    vector.tensor_scalar_mul(
                        carrysc_sb[:, cc * KD:(cc + 1) * KD],
                        carry_sb[:, (cc - 1) * KD:cc * KD],
                        float(2.0 ** -E_NT[cc % NCHUNK])).then_inc(cssem, 1)

                def res(cc):
                    """Chunk-0 NT residual: NTr08 = psum*2^E0N - float(NT08)."""
                    if cc >= NCHT or cc % NCHUNK != 0:
                        return
                    jc = it * NCHT + cc
                    bt = cc // NCHUNK
                    # RAW fence: the main chunk-0 NT write retired
                    vector.wait_ge(ctsbD, jc + 1)
                    vector.scalar_tensor_tensor(
                        ct8r[:, 0:KD, bt * P:(bt + 1) * P],
                        ctps[:].rearrange("p (k t) -> p k t", k=KD),
                        float(2.0 ** E0N),
                        ct8[:, 0:KD, cc * P:(cc + 1) * P],
                        Alu.mult, Alu.subtract).then_inc(ctrsem, 1)

                # walk the schedule: NT chain ops + DVE's share of evictions
                s = 0
                for op in _OPS:
                    if op[0] == "ntD":
                        ntD(op[1])
                    elif op[0] == "res":
                        res(op[1])
                    elif op[0] == "csc":
                        csc(op[1])
                    elif op[0] == "pair":
                        for ei in _ENTRY_EOPS[s]:
                            if _ENG_EOP[ei] == "d":
                                evict(it, ei)
                        s += 1

    return nc


@functools.lru_cache(maxsize=None)
def _get_program(has_bias: bool, reps: int = 1, dbg: bool = False):
    return _build(has_bias, reps, dbg)


@functools.lru_cache(maxsize=None)
def _host_consts():
    import ml_dtypes
    t = np.arange(T, dtype=np.float64)
    den = (t + 1.0) * (t + 2.0) / 2.0
    s = np.arange(P)
    tril_t = (s[:, None] <= s[None, :]).astype(np.float32)  # [s, t] s<=t
    c16 = np.zeros((P, NCHUNK * P), dtype=ml_dtypes.bfloat16)
    for c in range(NCHUNK):
        posw = (np.arange(c * P, (c + 1) * P, dtype=np.float32) + 1.0)
        utw = posw[:, None] * tril_t
        if c == 0:
            # chunk 0 has no carry, so its UTW is den-NORMALIZED: the psum
            # comes out as out-scale values, fp8-safe across the chunk
            utw = utw / den[None, 0:P]
        c16[:, c * P:(c + 1) * P] = utw.astype(ml_dtypes.bfloat16)
    c32 = np.zeros((P, NCHUNK), dtype=np.float32)
    c32[:, 0] = 2.0 ** (-E0N - EW)
    for c in range(1, NCHUNK):
        sc = 2.0 ** (E_NT[c] - EW + S_OUT[c])
        c32[:, c] = (sc / den[c * P:(c + 1) * P]).astype(np.float32)
    return c16, c32


def make_in_maps(context, emb_table, W, b):
    import ml_dtypes
    context = np.asarray(context)
    emb_table = np.asarray(emb_table, dtype=np.float32)
    W = np.asarray(W, dtype=np.float32)
    b = np.asarray(b, dtype=np.float32)
    has_bias = bool(np.any(b))

    table16 = np.ascontiguousarray(emb_table.astype(ml_dtypes.bfloat16))
    wts = W.T.astype(np.float32) * (2.0 ** EW)                       # (D, V)
    w8_full = wts.astype(ml_dtypes.float8_e4m3)
    wr8_full = (wts - w8_full.astype(np.float32)).astype(ml_dtypes.float8_e4m3)
    c16, c32 = _host_consts()

    def plane_pack(full, vg):
        w = np.zeros((P, NGRP * KD, GCOLS), dtype=ml_dtypes.float8_e4m3)
        for g in range(NGRP):
            for k in range(KD):
                w[:, g * KD + k, :] = full[
                    k * P:(k + 1) * P,
                    vg * V_CORE + g * GCOLS:vg * V_CORE + (g + 1) * GCOLS]
        return w.reshape(P, NGRP * KD * GCOLS)

    in_maps = []
    for ci in range(NCORE):
        vg, bg = ci % NVG, ci // NVG
        idx = np.concatenate(
            [context[bg * NB + bt].reshape(NCHUNK, P).T for bt in range(NB)],
            axis=1).astype(np.int32)           # [p, cc]
        in_maps.append({"idx": np.ascontiguousarray(idx), "table": table16,
                        "wt8": plane_pack(w8_full, vg),
                        "wr8": plane_pack(wr8_full, vg),
                        "cst16": c16, "cst32": c32})
    return in_maps, has_bias


def kernel(context, emb_table, W, b):
    in_maps, has_bias = make_in_maps(context, emb_table, W, b)
    nc = _get_program(False)
    res = None
    for attempt in range(3):
        try:
            res = run_bass_kernel_spmd(nc, in_maps, list(range(NCORE)))
            break
        except Exception:
            # transient NRT errors on the axon-tunneled device: retry
            if attempt == 2:
                raise
            import time
            time.sleep(10.0 * (attempt + 1))

    deq = np.array([2.0 ** -S_OUT[c] for c in range(1, NCHUNK)],
                   dtype=np.float32)[None, :, None, None]
    out = np.empty((B, T, V), dtype=np.float32)
    for ci in range(NCORE):
        vg, bg = ci % NVG, ci // NVG
        vsl = slice(vg * V_CORE, (vg + 1) * V_CORE)
        o16 = np.asarray(res.results[ci]["out16"]).astype(np.float32)
        o8 = (np.asarray(res.results[ci]["out8"]).astype(np.float32)
              .reshape(NB, NCHUNK - 1, P, V_CORE) * deq)
        for bt in range(NB):
            bfull = bg * NB + bt
            out[bfull, 0:P, vsl] = o16[bt * P:(bt + 1) * P]
            out[bfull, P:T, vsl] = o8[bt].reshape((NCHUNK - 1) * P, V_CORE)
    if has_bias:
        out += np.asarray(b, dtype=np.float32)[None, None, :]
    return out
